# revision 3
# baseline (speedup 1.0000x reference)
"""Distributed causal attention head on 8 TRN2 NeuronCores.

Problem: B=4, S=4096, D_in=512, D_out=64 causal attention
  K/V/Q = X @ W; scores = Q@K^T (causal, /sqrt(64)); Z = softmax(scores)@V

Sharding: core c = 2*b + h handles batch b, seq-half h.
q-rows are interleaved at 128-row-block granularity (core h owns global
q-blocks {2j+h}), which makes the causal block schedule IDENTICAL on all
cores (SPMD-safe) and balances FLOPs exactly.  Every core loads the full
(transposed) K/V inputs of its batch and projects them locally.

v2 perf structure:
 - PE warmup spin (dummy matmuls on the weight tile) so the HAM clock
   gate is at 8/8 (2.4 GHz) when real projections start.
 - Input DMAs are split so chunk-0's slices land first, and triggers are
   spread round-robin across the sync/vector/scalar/gpsimd queues
   (each trigger costs ~600ns on its issuing queue).
 - Wq/Wk are host-duplicated to [D, 128] so projections emit [128, 512]
   PSUM (both parity copies in one matmul + one CAST), feeding the
   row-tiled score matmuls directly.
 - Scores are computed transposed ST[k,q] with K=64 PAIRS row-tiled in
   the PE; exp on ACT in groups of 2 kblocks (scale=1/8 folded, no
   max-subtraction: |scores/8| < ~1.5); AV matmuls accumulate Z^T in
   PSUM with a ones-column in Vp giving the softmax denominator free;
   Z^T is PE-transposed back to q-major, normalized with a reciprocal +
   tensor_scalar_mul into a persistent output tile, DMA'd per chunk.
"""

import numpy as np
import ml_dtypes

import concourse.bass as bass
import concourse.bacc as bacc
import concourse.mybir as mybir
import concourse.tile as tile

B, S, D, E = 4, 4096, 512, 64
PB = 128                      # partition block
NKB = S // PB                 # 32 k-blocks (global)
NLQ = NKB // 2                # 16 local q-blocks per core
NCH = 4                       # q-chunks of 512 per core
CHW = 512                     # q-chunk width
ND = D // PB                  # 4 d-slices
GRP = 2                       # kblocks per exp group
LAG = 4                       # ST->AV software pipeline depth (groups)
NWARM = 10                    # PE warmup matmuls
BF16 = mybir.dt.bfloat16
F32 = mybir.dt.float32
NPBF16 = ml_dtypes.bfloat16


def build_nc():
    nc = bacc.Bacc(None)

    xq_d = nc.declare_dram_parameter("xq", [D, S // 2], BF16, isOutput=False)
    xk_d = nc.declare_dram_parameter("xk", [D, S], BF16, isOutput=False)
    xv_d = nc.declare_dram_parameter("xv", [D, S], BF16, isOutput=False)
    wq_d = nc.declare_dram_parameter("wq", [D, PB], BF16, isOutput=False)  # dup
    wk_d = nc.declare_dram_parameter("wk", [D, PB], BF16, isOutput=False)  # dup
    wv_d = nc.declare_dram_parameter("wv", [D, E], BF16, isOutput=False)
    cm_d = nc.declare_dram_parameter("cmask", [8, PB, CHW], BF16, isOutput=False)
    id_d = nc.declare_dram_parameter("ident", [PB, PB], F32, isOutput=False)
    out_d = nc.declare_dram_parameter("out", [S // 2, E], F32, isOutput=True)

    with tile.TileContext(nc) as tc:
        with tc.tile_pool(name="persist", bufs=1) as pp, \
             tc.tile_pool(name="st_ps", bufs=2, space="PSUM") as stp, \
             tc.tile_pool(name="pj_ps", bufs=2, space="PSUM") as pjp, \
             tc.tile_pool(name="zt_ps", bufs=2, space="PSUM") as ztp, \
             tc.tile_pool(name="work", bufs=2 * LAG + 2) as wp:
            # ---- persistent SBUF tiles ----
            wq_sb = pp.tile([PB, ND * PB], BF16, name="wq_sb", tag="wq_sb")
            wk_sb = pp.tile([PB, ND * PB], BF16, name="wk_sb", tag="wk_sb")
            wv_sb = pp.tile([PB, ND * E], BF16, name="wv_sb", tag="wv_sb")
            mk_sb = pp.tile([PB, 8 * CHW], BF16, name="mk_sb", tag="mk_sb")
            idf_sb = pp.tile([PB, PB], F32, name="idf_sb", tag="idf_sb")
            idb_sb = pp.tile([PB, PB], BF16, name="idb_sb", tag="idb_sb")
            # inputs: chunk-0 slices separate so the pipeline starts early
            xqA = [pp.tile([PB, CHW], BF16, name=f"xqA{d}", tag=f"xqA{d}")
                   for d in range(ND)]
            xqB = [pp.tile([PB, 3 * CHW], BF16, name=f"xqB{d}", tag=f"xqB{d}")
                   for d in range(ND)]
            xkA = [pp.tile([PB, 2 * CHW], BF16, name=f"xkA{d}", tag=f"xkA{d}")
                   for d in range(ND)]
            xkB = [pp.tile([PB, 6 * CHW], BF16, name=f"xkB{d}", tag=f"xkB{d}")
                   for d in range(ND)]
            xvA = [pp.tile([PB, 2 * CHW], BF16, name=f"xvA{d}", tag=f"xvA{d}")
                   for d in range(ND)]
            xvB = [pp.tile([PB, 6 * CHW], BF16, name=f"xvB{d}", tag=f"xvB{d}")
                   for d in range(ND)]
            # projected tensors, chunked  (rows 0:64 == rows 64:128)
            qpT = [pp.tile([PB, CHW], BF16, name=f"qpT{c}", tag=f"qpT{c}")
                   for c in range(NCH)]
            kpT = [pp.tile([PB, CHW], BF16, name=f"kpT{c}", tag=f"kpT{c}")
                   for c in range(2 * NCH)]
            vpT = [pp.tile([E, CHW], BF16, name=f"vpT{c}", tag=f"vpT{c}")
                   for c in range(2 * NCH)]
            vp = [pp.tile([PB, E + 1], BF16, name=f"vp{s}", tag=f"vp{s}")
                  for s in range(NKB)]
            out_sb = pp.tile([PB, NLQ * E], F32, name="out_sb", tag="out_sb")

            def xq_ap(d, c):
                return xqA[d][:] if c == 0 else xqB[d][:, CHW * (c - 1):CHW * c]

            def xk_ap(d, kc):
                return (xkA[d][:, CHW * kc:CHW * (kc + 1)] if kc < 2
                        else xkB[d][:, CHW * (kc - 2):CHW * (kc - 1)])

            def xv_ap(d, kc):
                return (xvA[d][:, CHW * kc:CHW * (kc + 1)] if kc < 2
                        else xvB[d][:, CHW * (kc - 2):CHW * (kc - 1)])

            # ---- input DMAs: priority order, round-robin over 4 queues ----
            dmas = []
            dmas.append((wq_sb[:].rearrange("p (d e) -> p d e", e=PB),
                         wq_d.rearrange("(d p) e -> p d e", p=PB)))
            dmas.append((wk_sb[:].rearrange("p (d e) -> p d e", e=PB),
                         wk_d.rearrange("(d p) e -> p d e", p=PB)))
            dmas.append((idf_sb[:], id_d[:]))
            for d in range(ND):
                dmas.append((xqA[d][:], xq_d[PB * d:PB * (d + 1), 0:CHW]))
            for d in range(ND):
                dmas.append((xkA[d][:], xk_d[PB * d:PB * (d + 1), 0:2 * CHW]))
            dmas.append((wv_sb[:].rearrange("p (d e) -> p d e", e=E),
                         wv_d.rearrange("(d p) e -> p d e", p=PB)))
            for d in range(ND):
                dmas.append((xvA[d][:], xv_d[PB * d:PB * (d + 1), 0:2 * CHW]))
            dmas.append((mk_sb[:].rearrange("p (m q) -> p m q", q=CHW),
                         cm_d.rearrange("m p q -> p m q")))
            for d in range(ND):
                dmas.append((xqB[d][:], xq_d[PB * d:PB * (d + 1), CHW:4 * CHW]))
            for d in range(ND):
                dmas.append((xkB[d][:], xk_d[PB * d:PB * (d + 1), 2 * CHW:8 * CHW]))
            for d in range(ND):
                dmas.append((xvB[d][:], xv_d[PB * d:PB * (d + 1), 2 * CHW:8 * CHW]))
            # early (critical) DMAs round-robin over 3 queues; later bulk
            # DMAs avoid the scalar queue (it runs the exp stream)
            qcrit = [nc.sync, nc.scalar, nc.gpsimd]
            qbulk = [nc.sync, nc.gpsimd]
            for i, (o, inp) in enumerate(dmas):
                q = qcrit[i % 3] if i < 17 else qbulk[i % 2]
                q.dma_start(out=o, in_=inp)

            nc.vector.tensor_copy(idb_sb[:], idf_sb[:])
            for s in range(NKB):
                nc.vector.memset(vp[s][:], 1.0)   # ones column prefill

            # ---- PE warmup: keep HAM busy until real projections start ----
            for i in range(NWARM):
                wm_ps = stp.tile([PB, CHW], F32, tag="st")
                nc.tensor.matmul(wm_ps[:], wq_sb[:, 0:PB], wq_sb[:, 0:4 * PB],
                                 start=True, stop=True)

            def vtrans(s):
                """PE-transpose one projected-V block to k-major + copy out."""
                vproj(s // 4)
                vt_ps = pjp.tile([PB, E], BF16, tag="pj")
                nc.tensor.transpose(vt_ps[:], vpT[s // 4][:, PB * (s % 4):PB * (s % 4 + 1)],
                                    idb_sb[0:E, 0:E])
                nc.vector.tensor_copy(vp[s][:, 0:E], vt_ps[:])

            def project(c):
                """Project Q chunk c and K chunks 2c, 2c+1 (V projections
                are emitted later, interleaved between ST groups)."""
                qp_ps = pjp.tile([PB, CHW], F32, tag="pj")
                for d in range(ND):
                    nc.tensor.matmul(qp_ps[:], wq_sb[:, PB * d:PB * (d + 1)],
                                     xq_ap(d, c),
                                     start=(d == 0), stop=(d == ND - 1))
                nc.vector.tensor_copy(qpT[c][:], qp_ps[:])
                for kc in (2 * c, 2 * c + 1):
                    kp_ps = pjp.tile([PB, CHW], F32, tag="pj")
                    for d in range(ND):
                        nc.tensor.matmul(kp_ps[:], wk_sb[:, PB * d:PB * (d + 1)],
                                         xk_ap(d, kc),
                                         start=(d == 0), stop=(d == ND - 1))
                    nc.vector.tensor_copy(kpT[kc][:], kp_ps[:])

            vproj_done = set()

            def vproj(kc):
                """Lazily project V chunk kc (called at first vtrans use)."""
                if kc in vproj_done:
                    return
                vproj_done.add(kc)
                vq_ps = pjp.tile([E, CHW], F32, tag="pj")
                for d in range(ND):
                    nc.tensor.matmul(vq_ps[:], wv_sb[:, E * d:E * (d + 1)],
                                     xv_ap(d, kc),
                                     start=(d == 0), stop=(d == ND - 1))
                nc.vector.tensor_copy(vpT[kc][:], vq_ps[:])

            def st_mm(st_ps, ji, kb, c):
                pb = E * (kb % 2)
                kc, col = kb // 4, PB * (kb % 4)
                nc.tensor.matmul(st_ps[:, CHW * ji:CHW * (ji + 1)],
                                 kpT[kc][pb:pb + E, col:col + PB],
                                 qpT[c][pb:pb + E, :],
                                 start=True, stop=True, tile_position=(pb, 0))

            # prologue: first chunk's projections
            project(0)

            for c in range(NCH):
                nkb = 8 * c + 8
                zt_ps = ztp.tile([E + 1, CHW], F32, tag="zt")
                korder = list(range(0, nkb))
                groups = [korder[i:i + GRP] for i in range(0, nkb, GRP)]
                pend = []
                drain_state = {"n": 0}

                def drain_avs(p_et, p_kbs, nkb=nkb, zt_ps=zt_ps, c=c, ds=drain_state):
                    for kb in p_kbs:      # late vtrans, spread across groups
                        if kb >= 8 * c:
                            vtrans(kb)
                    for ji, kb in enumerate(p_kbs):
                        nc.tensor.matmul(
                            zt_ps[:], vp[kb][:],
                            p_et[:, CHW * ji:CHW * (ji + 1)],
                            start=(ds["n"] == 0),
                            stop=(ds["n"] == nkb - 1),
                            skip_group_check=True)
                        ds["n"] += 1

                for gi, kbs in enumerate(groups):
                    gw = len(kbs) * CHW
                    st_ps = stp.tile([PB, GRP * CHW], F32, tag="st")
                    # pair of consecutive kblocks -> concurrent row-tiled MMs
                    if len(kbs) >= 2:
                        st_mm(st_ps, 0, kbs[0], c)
                        st_mm(st_ps, 1, kbs[1], c)
                        rest = range(2, len(kbs))
                    else:
                        rest = range(len(kbs))
                    for ji in rest:
                        st_mm(st_ps, ji, kbs[ji], c)
                    if len(pend) > LAG - 1:
                        drain_avs(*pend.pop(0))
                    et_sb = wp.tile([PB, GRP * CHW], BF16, tag="et")
                    nc.scalar.activation(
                        et_sb[:, :gw], st_ps[:, :gw],
                        mybir.ActivationFunctionType.Exp, scale=0.125)
                    for ji, kb in enumerate(kbs):
                        m = kb - 8 * c
                        if m >= 0:
                            nc.vector.tensor_mul(
                                et_sb[:, CHW * ji:CHW * (ji + 1)],
                                et_sb[:, CHW * ji:CHW * (ji + 1)],
                                mk_sb[:, CHW * m:CHW * (m + 1)])
                    pend.append((et_sb, kbs))
                for p in pend:
                    drain_avs(*p)
                zs_sb = wp.tile([E + 1, CHW], F32, tag="zs")
                nc.vector.tensor_copy(zs_sb[:], zt_ps[:])
                # project next chunk while exp/AV tail of this chunk drains
                if c + 1 < NCH:
                    project(c + 1)
                # normalize via transpose (denominator = col E)
                for j in range(4):
                    zn_ps = ztp.tile([PB, E + 1], F32, tag="zt")
                    nc.tensor.transpose(zn_ps[:], zs_sb[:, PB * j:PB * (j + 1)],
                                        idf_sb[0:E + 1, 0:E + 1])
                    rc_sb = wp.tile([PB, 1], F32, tag="rc")
                    nc.vector.reciprocal(rc_sb[:], zn_ps[:, E:E + 1])
                    jj = 4 * c + j
                    nc.vector.tensor_scalar_mul(out_sb[:, E * jj:E * (jj + 1)],
                                                zn_ps[:, 0:E], rc_sb[:])
                # chunk's output block: one DMA of [128, 4*E]
                nc.gpsimd.dma_start(
                    out=out_d[CHW * c:CHW * (c + 1), :].rearrange(
                        "(j p) e -> p j e", p=PB),
                    in_=out_sb[:, 4 * E * c:4 * E * (c + 1)].rearrange(
                        "p (j e) -> p j e", e=E))
    nc.finalize()
    return nc


def make_core_inputs(key_np, value_np, query_np, Wk, Wv, Wq):
    """Host-side sharding: returns in_maps list of 8 dicts."""
    bf = lambda a: np.ascontiguousarray(a).astype(NPBF16)
    wq2 = np.concatenate([Wq, Wq], axis=1)   # [D, 128] duplicated
    wk2 = np.concatenate([Wk, Wk], axis=1)
    in_maps = []
    for c in range(8):
        b, h = c // 2, c % 2
        qrows = np.concatenate(
            [np.arange(PB * (2 * j + h), PB * (2 * j + h) + PB) for j in range(NLQ)])
        # causal masks: mask m applies to kblock kb = 8c+m of every chunk;
        # section jj (q sub-block) has global q-block g = 8c+2jj+h,
        # class = m - 2jj - h: <0 keep, ==0 triangular, >0 zero.
        cmask = np.zeros((8, PB, CHW), dtype=np.float32)
        ki = np.arange(PB)[:, None]
        qi = np.arange(PB)[None, :]
        tri = (ki <= qi).astype(np.float32)
        for m in range(8):
            for jj in range(4):
                cls = m - 2 * jj - h
                blk = np.ones((PB, PB), np.float32) if cls < 0 else (
                    tri if cls == 0 else np.zeros((PB, PB), np.float32))
                cmask[m][:, PB * jj:PB * (jj + 1)] = blk
        in_maps.append({
            "xq": bf(query_np[b][qrows].T),
            "xk": bf(key_np[b].T),
            "xv": bf(value_np[b].T),
            "wq": bf(wq2), "wk": bf(wk2), "wv": bf(Wv),
            "cmask": bf(cmask),
            "ident": np.eye(PB, dtype=np.float32),
        })
    return in_maps


def assemble_output(results):
    """results: list of 8 dicts with 'out' [2048, 64] f32 -> Z [B,S,E]."""
    Z = np.zeros((B, S, E), dtype=np.float32)
    for c in range(8):
        b, h = c // 2, c % 2
        o = results[c]["out"]  # [2048, E] q-major
        for j in range(NLQ):
            g = 2 * j + h
            Z[b, PB * g:PB * (g + 1), :] = o[PB * j:PB * (j + 1), :]
    return Z


def kernel(key_inputs, value_inputs, query_inputs, Wk, Wv, Wq):
    from concourse.bass_utils import run_bass_kernel_spmd
    nc = build_nc()
    in_maps = make_core_inputs(np.asarray(key_inputs), np.asarray(value_inputs),
                               np.asarray(query_inputs), np.asarray(Wk),
                               np.asarray(Wv), np.asarray(Wq))
    res = run_bass_kernel_spmd(nc, in_maps, core_ids=list(range(8)))
    return assemble_output(res.results)


# revision 5
# speedup vs baseline: 1.0030x; 1.0030x over previous
"""Distributed causal attention head on 8 TRN2 NeuronCores.

Problem: B=4, S=4096, D_in=512, D_out=64 causal attention
  K/V/Q = X @ W; scores = Q@K^T (causal, /sqrt(64)); Z = softmax(scores)@V

Sharding: core c = 2*b + h handles batch b, seq-half h.
q-rows are interleaved at 128-row-block granularity (core h owns global
q-blocks {2j+h}), which makes the causal block schedule IDENTICAL on all
cores (SPMD-safe) and balances FLOPs exactly.  Every core loads the full
(transposed) K/V inputs of its batch and projects them locally.

Perf structure:
 - All host tensors are partition-major so every DMA descriptor is a
   1-8KB contiguous row (descriptor-processing, not bandwidth, limits
   badly-shaped DMAs).
 - Input DMAs are ordered by first-use time and split so each chunk's
   slices arrive just ahead of the compute that needs them; triggers are
   spread over the sync/scalar/gpsimd queues (a trigger occupies its
   queue ~0.6us and a queue's transfers serialize).
 - PE warmup spin (dummy matmuls on the weight tile) so the HAM clock
   gate is at 8/8 (2.4 GHz) when real projections start.
 - Wq/Wk are host-duplicated to [D, 128] so projections emit [128, 512]
   PSUM (both parity copies in one matmul + one CAST), feeding the
   row-tiled score matmuls directly.
 - Scores are computed transposed ST[k,q] with K=64 PAIRS row-tiled in
   the PE; exp on ACT in groups of 2 kblocks (scale=1/8 folded, no
   max-subtraction: |scores/8| < ~1.5); AV matmuls accumulate Z^T in
   PSUM with a ones-column in Vp giving the softmax denominator free;
   Z^T is PE-transposed back to q-major, normalized with a reciprocal +
   tensor_scalar_mul into a persistent [128, 16*64] output tile that is
   DMA'd per chunk (contiguous, host un-permutes).
"""

import numpy as np
import ml_dtypes

import concourse.bass as bass
import concourse.bacc as bacc
import concourse.mybir as mybir
import concourse.tile as tile

B, S, D, E = 4, 4096, 512, 64
PB = 128                      # partition block
NKB = S // PB                 # 32 k-blocks (global)
NLQ = NKB // 2                # 16 local q-blocks per core
NCH = 4                       # q-chunks of 512 per core
CHW = 512                     # q-chunk width
ND = D // PB                  # 4 d-slices
GRP = 2                       # kblocks per exp group
LAG = 4                       # ST->AV software pipeline depth (groups)
NWARM = 12                    # PE warmup matmuls
BF16 = mybir.dt.bfloat16
F32 = mybir.dt.float32
NPBF16 = ml_dtypes.bfloat16


def build_nc():
    nc = bacc.Bacc(None)

    xq_d = nc.declare_dram_parameter("xq", [D, S // 2], BF16, isOutput=False)
    xk_d = nc.declare_dram_parameter("xk", [D, S], BF16, isOutput=False)
    xv_d = nc.declare_dram_parameter("xv", [D, S], BF16, isOutput=False)
    wq_d = nc.declare_dram_parameter("wq", [PB, ND * PB], BF16, isOutput=False)
    wk_d = nc.declare_dram_parameter("wk", [PB, ND * PB], BF16, isOutput=False)
    wv_d = nc.declare_dram_parameter("wv", [PB, ND * E], BF16, isOutput=False)
    cm_d = nc.declare_dram_parameter("cmask", [PB, 8 * CHW], BF16, isOutput=False)
    id_d = nc.declare_dram_parameter("ident", [PB, PB], F32, isOutput=False)
    out_d = nc.declare_dram_parameter("out", [PB, NLQ * E], F32, isOutput=True)

    with tile.TileContext(nc) as tc:
        with tc.tile_pool(name="persist", bufs=1) as pp, \
             tc.tile_pool(name="st_ps", bufs=2, space="PSUM") as stp, \
             tc.tile_pool(name="pj_ps", bufs=2, space="PSUM") as pjp, \
             tc.tile_pool(name="zt_ps", bufs=2, space="PSUM") as ztp, \
             tc.tile_pool(name="work", bufs=2 * LAG + 2) as wp:
            # ---- persistent SBUF tiles ----
            wq_sb = pp.tile([PB, ND * PB], BF16, name="wq_sb", tag="wq_sb")
            wk_sb = pp.tile([PB, ND * PB], BF16, name="wk_sb", tag="wk_sb")
            wv_sb = pp.tile([PB, ND * E], BF16, name="wv_sb", tag="wv_sb")
            mk_sb = pp.tile([PB, 8 * CHW], BF16, name="mk_sb", tag="mk_sb")
            idf_sb = pp.tile([PB, PB], F32, name="idf_sb", tag="idf_sb")
            idb_sb = pp.tile([PB, PB], BF16, name="idb_sb", tag="idb_sb")
            # inputs, split by first-use time (A: chunk 0, B1: chunk 1,
            # B2: chunks 2-3 / k-chunks 4-7)
            xqA = [pp.tile([PB, CHW], BF16, name=f"xqA{d}", tag=f"xqA{d}")
                   for d in range(ND)]
            xqB1 = [pp.tile([PB, CHW], BF16, name=f"xqB1{d}", tag=f"xqB1{d}")
                    for d in range(ND)]
            xqB2 = [pp.tile([PB, 2 * CHW], BF16, name=f"xqB2{d}", tag=f"xqB2{d}")
                    for d in range(ND)]
            xkA = [pp.tile([PB, 2 * CHW], BF16, name=f"xkA{d}", tag=f"xkA{d}")
                   for d in range(ND)]
            xkB1 = [pp.tile([PB, 2 * CHW], BF16, name=f"xkB1{d}", tag=f"xkB1{d}")
                    for d in range(ND)]
            xkB2 = [pp.tile([PB, 4 * CHW], BF16, name=f"xkB2{d}", tag=f"xkB2{d}")
                    for d in range(ND)]
            xvA = [pp.tile([PB, 2 * CHW], BF16, name=f"xvA{d}", tag=f"xvA{d}")
                   for d in range(ND)]
            xvB1 = [pp.tile([PB, 2 * CHW], BF16, name=f"xvB1{d}", tag=f"xvB1{d}")
                    for d in range(ND)]
            xvB2 = [pp.tile([PB, 4 * CHW], BF16, name=f"xvB2{d}", tag=f"xvB2{d}")
                    for d in range(ND)]
            # projected tensors, chunked  (rows 0:64 == rows 64:128)
            qpT = [pp.tile([PB, CHW], BF16, name=f"qpT{c}", tag=f"qpT{c}")
                   for c in range(NCH)]
            kpT = [pp.tile([PB, CHW], BF16, name=f"kpT{c}", tag=f"kpT{c}")
                   for c in range(2 * NCH)]
            vpT = [pp.tile([E, CHW], BF16, name=f"vpT{c}", tag=f"vpT{c}")
                   for c in range(2 * NCH)]
            vp = [pp.tile([PB, E + 1], BF16, name=f"vp{s}", tag=f"vp{s}")
                  for s in range(NKB)]
            out_sb = pp.tile([PB, NLQ * E], F32, name="out_sb", tag="out_sb")

            def xq_ap(d, c):
                if c == 0:
                    return xqA[d][:]
                if c == 1:
                    return xqB1[d][:]
                return xqB2[d][:, CHW * (c - 2):CHW * (c - 1)]

            def xk_ap(d, kc):
                if kc < 2:
                    return xkA[d][:, CHW * kc:CHW * (kc + 1)]
                if kc < 4:
                    return xkB1[d][:, CHW * (kc - 2):CHW * (kc - 1)]
                return xkB2[d][:, CHW * (kc - 4):CHW * (kc - 3)]

            def xv_ap(d, kc):
                if kc < 2:
                    return xvA[d][:, CHW * kc:CHW * (kc + 1)]
                if kc < 4:
                    return xvB1[d][:, CHW * (kc - 2):CHW * (kc - 1)]
                return xvB2[d][:, CHW * (kc - 4):CHW * (kc - 3)]

            # ---- input DMAs: by first-use time, explicit queues ----
            def xrow(t, lo, hi):
                return (lambda d: t[PB * d:PB * (d + 1), lo:hi])
            xq_r = xrow(xq_d, 0, 0)  # placeholder (unused)
            dma_scalar = [
                (wq_sb[:], wq_d[:]),
                (idf_sb[:], id_d[:]),
                (xqA[3][:], xq_d[3 * PB:4 * PB, 0:CHW]),
                (xkA[3][:], xk_d[3 * PB:4 * PB, 0:2 * CHW]),
                (wv_sb[:], wv_d[:]),
                (mk_sb[:], cm_d[:]),
            ]
            dma_sync = [
                (wk_sb[:], wk_d[:]),
                (xqA[0][:], xq_d[0:PB, 0:CHW]),
                (xkA[0][:], xk_d[0:PB, 0:2 * CHW]),
                (xqA[1][:], xq_d[PB:2 * PB, 0:CHW]),
                (xkA[1][:], xk_d[PB:2 * PB, 0:2 * CHW]),
                (xvA[0][:], xv_d[0:PB, 0:2 * CHW]),
                (xvA[1][:], xv_d[PB:2 * PB, 0:2 * CHW]),
                (xqB1[0][:], xq_d[0:PB, CHW:2 * CHW]),
                (xqB1[1][:], xq_d[PB:2 * PB, CHW:2 * CHW]),
                (xkB1[0][:], xk_d[0:PB, 2 * CHW:4 * CHW]),
                (xkB1[1][:], xk_d[PB:2 * PB, 2 * CHW:4 * CHW]),
                (xvB1[0][:], xv_d[0:PB, 2 * CHW:4 * CHW]),
                (xvB1[1][:], xv_d[PB:2 * PB, 2 * CHW:4 * CHW]),
                (xqB2[0][:], xq_d[0:PB, 2 * CHW:4 * CHW]),
                (xqB2[1][:], xq_d[PB:2 * PB, 2 * CHW:4 * CHW]),
                (xkB2[0][:], xk_d[0:PB, 4 * CHW:8 * CHW]),
                (xkB2[1][:], xk_d[PB:2 * PB, 4 * CHW:8 * CHW]),
                (xvB2[0][:], xv_d[0:PB, 4 * CHW:8 * CHW]),
                (xvB2[1][:], xv_d[PB:2 * PB, 4 * CHW:8 * CHW]),
            ]
            dma_pool = [
                (xqA[2][:], xq_d[2 * PB:3 * PB, 0:CHW]),
                (xkA[2][:], xk_d[2 * PB:3 * PB, 0:2 * CHW]),
                (xvA[2][:], xv_d[2 * PB:3 * PB, 0:2 * CHW]),
                (xvA[3][:], xv_d[3 * PB:4 * PB, 0:2 * CHW]),
                (xqB1[2][:], xq_d[2 * PB:3 * PB, CHW:2 * CHW]),
                (xqB1[3][:], xq_d[3 * PB:4 * PB, CHW:2 * CHW]),
                (xkB1[2][:], xk_d[2 * PB:3 * PB, 2 * CHW:4 * CHW]),
                (xkB1[3][:], xk_d[3 * PB:4 * PB, 2 * CHW:4 * CHW]),
                (xvB1[2][:], xv_d[2 * PB:3 * PB, 2 * CHW:4 * CHW]),
                (xvB1[3][:], xv_d[3 * PB:4 * PB, 2 * CHW:4 * CHW]),
                (xqB2[2][:], xq_d[2 * PB:3 * PB, 2 * CHW:4 * CHW]),
                (xqB2[3][:], xq_d[3 * PB:4 * PB, 2 * CHW:4 * CHW]),
                (xkB2[2][:], xk_d[2 * PB:3 * PB, 4 * CHW:8 * CHW]),
                (xkB2[3][:], xk_d[3 * PB:4 * PB, 4 * CHW:8 * CHW]),
                (xvB2[2][:], xv_d[2 * PB:3 * PB, 4 * CHW:8 * CHW]),
                (xvB2[3][:], xv_d[3 * PB:4 * PB, 4 * CHW:8 * CHW]),
            ]
            for o, inp in dma_scalar:
                nc.scalar.dma_start(out=o, in_=inp)
            for o, inp in dma_sync:
                nc.sync.dma_start(out=o, in_=inp)
            for o, inp in dma_pool:
                nc.gpsimd.dma_start(out=o, in_=inp)

            nc.vector.tensor_copy(idb_sb[:], idf_sb[:])
            for s in range(NKB):
                nc.vector.memset(vp[s][:], 1.0)   # ones column prefill

            # ---- PE warmup: keep HAM busy until real projections start ----
            for i in range(NWARM):
                wm_ps = stp.tile([PB, CHW], F32, tag="st")
                nc.tensor.matmul(wm_ps[:], wq_sb[:, 0:PB], wq_sb[:, 0:4 * PB],
                                 start=True, stop=True)

            def vtrans(s):
                """PE-transpose one projected-V block to k-major + copy out."""
                vproj(s // 4)
                vt_ps = pjp.tile([PB, E], BF16, tag="pj")
                nc.tensor.transpose(vt_ps[:], vpT[s // 4][:, PB * (s % 4):PB * (s % 4 + 1)],
                                    idb_sb[0:E, 0:E])
                nc.vector.tensor_copy(vp[s][:, 0:E], vt_ps[:])

            def project(c):
                """Project Q chunk c and K chunks 2c, 2c+1 (V projections
                are emitted later, interleaved between ST groups)."""
                qp_ps = pjp.tile([PB, CHW], F32, tag="pj")
                for d in range(ND):
                    nc.tensor.matmul(qp_ps[:], wq_sb[:, PB * d:PB * (d + 1)],
                                     xq_ap(d, c),
                                     start=(d == 0), stop=(d == ND - 1))
                nc.vector.tensor_copy(qpT[c][:], qp_ps[:])
                for kc in (2 * c, 2 * c + 1):
                    kp_ps = pjp.tile([PB, CHW], F32, tag="pj")
                    for d in range(ND):
                        nc.tensor.matmul(kp_ps[:], wk_sb[:, PB * d:PB * (d + 1)],
                                         xk_ap(d, kc),
                                         start=(d == 0), stop=(d == ND - 1))
                    nc.vector.tensor_copy(kpT[kc][:], kp_ps[:])

            vproj_done = set()

            def vproj(kc):
                """Lazily project V chunk kc (called at first vtrans use)."""
                if kc in vproj_done:
                    return
                vproj_done.add(kc)
                vq_ps = pjp.tile([E, CHW], F32, tag="pj")
                for d in range(ND):
                    nc.tensor.matmul(vq_ps[:], wv_sb[:, E * d:E * (d + 1)],
                                     xv_ap(d, kc),
                                     start=(d == 0), stop=(d == ND - 1))
                nc.vector.tensor_copy(vpT[kc][:], vq_ps[:])

            def st_mm(st_ps, ji, kb, c):
                pb = E * (kb % 2)
                kc, col = kb // 4, PB * (kb % 4)
                nc.tensor.matmul(st_ps[:, CHW * ji:CHW * (ji + 1)],
                                 kpT[kc][pb:pb + E, col:col + PB],
                                 qpT[c][pb:pb + E, :],
                                 start=True, stop=True, tile_position=(pb, 0))

            # prologue: first chunk's projections
            project(0)

            for c in range(NCH):
                nkb = 8 * c + 8
                zt_ps = ztp.tile([E + 1, CHW], F32, tag="zt")
                korder = list(range(0, nkb))
                groups = [korder[i:i + GRP] for i in range(0, nkb, GRP)]
                pend = []
                drain_state = {"n": 0}

                def drain_avs(p_et, p_kbs, nkb=nkb, zt_ps=zt_ps, c=c, ds=drain_state):
                    for kb in p_kbs:      # late vtrans, spread across groups
                        if kb >= 8 * c:
                            vtrans(kb)
                    for ji, kb in enumerate(p_kbs):
                        nc.tensor.matmul(
                            zt_ps[:], vp[kb][:],
                            p_et[:, CHW * ji:CHW * (ji + 1)],
                            start=(ds["n"] == 0),
                            stop=(ds["n"] == nkb - 1),
                            skip_group_check=True)
                        ds["n"] += 1

                for gi, kbs in enumerate(groups):
                    gw = len(kbs) * CHW
                    st_ps = stp.tile([PB, GRP * CHW], F32, tag="st")
                    # pair of consecutive kblocks -> concurrent row-tiled MMs
                    if len(kbs) >= 2:
                        st_mm(st_ps, 0, kbs[0], c)
                        st_mm(st_ps, 1, kbs[1], c)
                        rest = range(2, len(kbs))
                    else:
                        rest = range(len(kbs))
                    for ji in rest:
                        st_mm(st_ps, ji, kbs[ji], c)
                    if len(pend) > LAG - 1:
                        drain_avs(*pend.pop(0))
                    et_sb = wp.tile([PB, GRP * CHW], BF16, tag="et")
                    nc.scalar.activation(
                        et_sb[:, :gw], st_ps[:, :gw],
                        mybir.ActivationFunctionType.Exp, scale=0.125)
                    m0 = kbs[0] - 8 * c
                    if len(kbs) == 2 and m0 >= 0:
                        # both kblocks in the diag region: adjacent masks,
                        # one [128, 1024] multiply
                        nc.vector.tensor_mul(
                            et_sb[:, :gw], et_sb[:, :gw],
                            mk_sb[:, CHW * m0:CHW * (m0 + 2)])
                    else:
                        for ji, kb in enumerate(kbs):
                            m = kb - 8 * c
                            if m >= 0:
                                nc.vector.tensor_mul(
                                    et_sb[:, CHW * ji:CHW * (ji + 1)],
                                    et_sb[:, CHW * ji:CHW * (ji + 1)],
                                    mk_sb[:, CHW * m:CHW * (m + 1)])
                    pend.append((et_sb, kbs))
                for p in pend:
                    drain_avs(*p)
                zs_sb = wp.tile([E + 1, CHW], F32, tag="zs")
                nc.vector.tensor_copy(zs_sb[:], zt_ps[:])
                # project next chunk while exp/AV tail of this chunk drains
                if c + 1 < NCH:
                    project(c + 1)
                # normalize via transpose (denominator = col E)
                for j in range(4):
                    zn_ps = ztp.tile([PB, E + 1], F32, tag="zt")
                    nc.tensor.transpose(zn_ps[:], zs_sb[:, PB * j:PB * (j + 1)],
                                        idf_sb[0:E + 1, 0:E + 1])
                    rc_sb = wp.tile([PB, 1], F32, tag="rc")
                    nc.vector.reciprocal(rc_sb[:], zn_ps[:, E:E + 1])
                    jj = 4 * c + j
                    nc.vector.tensor_scalar_mul(out_sb[:, E * jj:E * (jj + 1)],
                                                zn_ps[:, 0:E], rc_sb[:])
                # chunk's output block: one contiguous DMA of [128, 4*E] f32
                nc.gpsimd.dma_start(
                    out=out_d[:, 4 * E * c:4 * E * (c + 1)],
                    in_=out_sb[:, 4 * E * c:4 * E * (c + 1)])
    nc.finalize()
    return nc


def make_core_inputs(key_np, value_np, query_np, Wk, Wv, Wq):
    """Host-side sharding: returns in_maps list of 8 dicts."""
    bf = lambda a: np.ascontiguousarray(a).astype(NPBF16)

    def pmajor(w, width):
        """[D, width] -> [128, ND*width] partition-major."""
        return np.ascontiguousarray(
            w.reshape(ND, PB, width).transpose(1, 0, 2).reshape(PB, ND * width))

    wq2 = pmajor(np.concatenate([Wq, Wq], axis=1), PB)
    wk2 = pmajor(np.concatenate([Wk, Wk], axis=1), PB)
    wv2 = pmajor(Wv, E)
    in_maps = []
    for c in range(8):
        b, h = c // 2, c % 2
        qrows = np.concatenate(
            [np.arange(PB * (2 * j + h), PB * (2 * j + h) + PB) for j in range(NLQ)])
        # causal masks: mask m applies to kblock kb = 8c+m of every chunk;
        # section jj (q sub-block) has global q-block g = 8c+2jj+h,
        # class = m - 2jj - h: <0 keep, ==0 triangular, >0 zero.
        cmask = np.zeros((8, PB, CHW), dtype=np.float32)
        ki = np.arange(PB)[:, None]
        qi = np.arange(PB)[None, :]
        tri = (ki <= qi).astype(np.float32)
        for m in range(8):
            for jj in range(4):
                cls = m - 2 * jj - h
                blk = np.ones((PB, PB), np.float32) if cls < 0 else (
                    tri if cls == 0 else np.zeros((PB, PB), np.float32))
                cmask[m][:, PB * jj:PB * (jj + 1)] = blk
        cmask_pm = np.ascontiguousarray(
            cmask.transpose(1, 0, 2).reshape(PB, 8 * CHW))
        in_maps.append({
            "xq": bf(query_np[b][qrows].T),
            "xk": bf(key_np[b].T),
            "xv": bf(value_np[b].T),
            "wq": bf(wq2), "wk": bf(wk2), "wv": bf(wv2),
            "cmask": bf(cmask_pm),
            "ident": np.eye(PB, dtype=np.float32),
        })
    return in_maps


def assemble_output(results):
    """results: list of 8 dicts with 'out' [128, 16*64] f32 -> Z [B,S,E]."""
    Z = np.zeros((B, S, E), dtype=np.float32)
    for c in range(8):
        b, h = c // 2, c % 2
        o = results[c]["out"].reshape(PB, NLQ, E)  # [p, j, e]
        for j in range(NLQ):
            g = 2 * j + h
            Z[b, PB * g:PB * (g + 1), :] = o[:, j, :]
    return Z


def kernel(key_inputs, value_inputs, query_inputs, Wk, Wv, Wq):
    from concourse.bass_utils import run_bass_kernel_spmd
    nc = build_nc()
    in_maps = make_core_inputs(np.asarray(key_inputs), np.asarray(value_inputs),
                               np.asarray(query_inputs), np.asarray(Wk),
                               np.asarray(Wv), np.asarray(Wq))
    res = run_bass_kernel_spmd(nc, in_maps, core_ids=list(range(8)))
    return assemble_output(res.results)


# revision 8
# speedup vs baseline: 1.1741x; 1.1706x over previous
"""Distributed causal attention head on 8 TRN2 NeuronCores.

Problem: B=4, S=4096, D_in=512, D_out=64 causal attention
  K/V/Q = X @ W; scores = Q@K^T (causal, /sqrt(64)); Z = softmax(scores)@V

Sharding: core c = 2*b + h handles batch b, seq-half h.
q-rows are interleaved at 128-row-block granularity (core h owns global
q-blocks {2j+h}), which makes the causal block schedule IDENTICAL on all
cores (SPMD-safe) and balances FLOPs exactly.  Every core loads the full
(transposed) K/V inputs of its batch and projects them locally.

Perf structure:
 - All host tensors are partition-major so every DMA descriptor is a
   1-8KB contiguous row (descriptor-processing, not bandwidth, limits
   badly-shaped DMAs).
 - Input DMAs are ordered by first-use time and split so each chunk's
   slices arrive just ahead of the compute that needs them; triggers are
   spread over the sync/scalar/gpsimd queues (a trigger occupies its
   queue ~0.6us and a queue's transfers serialize).
 - PE warmup spin (dummy matmuls on the weight tile) so the HAM clock
   gate is at 8/8 (2.4 GHz) when real projections start.
 - Wq/Wk are host-duplicated to [D, 128] so projections emit [128, 512]
   PSUM (both parity copies in one matmul + one CAST), feeding the
   row-tiled score matmuls directly.
 - Scores are computed transposed ST[k,q] with K=64 PAIRS row-tiled in
   the PE; exp on ACT in groups of 2 kblocks (scale=1/8 folded, no
   max-subtraction: |scores/8| < ~1.5); AV matmuls accumulate Z^T in
   PSUM with a ones-column in Vp giving the softmax denominator free;
   Z^T is PE-transposed back to q-major, normalized with a reciprocal +
   tensor_scalar_mul into a persistent [128, 16*64] output tile that is
   DMA'd per chunk (contiguous, host un-permutes).
"""

import numpy as np
import ml_dtypes

import concourse.bass as bass
import concourse.bacc as bacc
import concourse.mybir as mybir
import concourse.tile as tile

B, S, D, E = 4, 4096, 512, 64
PB = 128                      # partition block
NKB = S // PB                 # 32 k-blocks (global)
NLQ = NKB // 2                # 16 local q-blocks per core
NCH = 4                       # q-chunks of 512 per core
CHW = 512                     # q-chunk width
ND = D // PB                  # 4 d-slices
GRP = 2                       # kblocks per exp group
LAG = 4                       # ST->AV software pipeline depth (groups)
NWARM = 10                    # PE warmup matmuls
BF16 = mybir.dt.bfloat16
F32 = mybir.dt.float32
NPBF16 = ml_dtypes.bfloat16


def build_nc():
    nc = bacc.Bacc(None)

    xq_d = nc.declare_dram_parameter("xq", [D, S // 2], BF16, isOutput=False)
    xk_d = nc.declare_dram_parameter("xk", [D, S], BF16, isOutput=False)
    xv_d = nc.declare_dram_parameter("xv", [D, S], BF16, isOutput=False)
    wq_d = nc.declare_dram_parameter("wq", [PB, ND * PB], BF16, isOutput=False)
    wk_d = nc.declare_dram_parameter("wk", [PB, ND * PB], BF16, isOutput=False)
    wv_d = nc.declare_dram_parameter("wv", [PB, ND * E], BF16, isOutput=False)
    cm_d = nc.declare_dram_parameter("cmask", [PB, 8 * CHW], BF16, isOutput=False)
    id_d = nc.declare_dram_parameter("ident", [PB, PB], F32, isOutput=False)
    out_d = nc.declare_dram_parameter("out", [PB, NLQ * E], F32, isOutput=True)

    with tile.TileContext(nc) as tc:
        with tc.tile_pool(name="persist", bufs=1) as pp, \
             tc.tile_pool(name="st_ps", bufs=2, space="PSUM") as stp, \
             tc.tile_pool(name="pj_ps", bufs=2, space="PSUM") as pjp, \
             tc.tile_pool(name="zt_ps", bufs=2, space="PSUM") as ztp, \
             tc.tile_pool(name="work", bufs=2 * LAG + 2) as wp:
            # ---- persistent SBUF tiles ----
            wq_sb = pp.tile([PB, ND * PB], BF16, name="wq_sb", tag="wq_sb")
            wk_sb = pp.tile([PB, ND * PB], BF16, name="wk_sb", tag="wk_sb")
            wv_sb = pp.tile([PB, ND * E], BF16, name="wv_sb", tag="wv_sb")
            mk_sb = pp.tile([PB, 8 * CHW], BF16, name="mk_sb", tag="mk_sb")
            idf_sb = pp.tile([PB, PB], F32, name="idf_sb", tag="idf_sb")
            idb_sb = pp.tile([PB, PB], BF16, name="idb_sb", tag="idb_sb")
            # inputs, split by first-use time (A: chunk 0, B1: chunk 1,
            # B2: chunks 2-3 / k-chunks 4-7); each tile holds all 4
            # d-slices side by side so one DMA covers it.
            xqA = pp.tile([PB, ND * CHW], BF16, name="xqA", tag="xqA")
            xqB1 = pp.tile([PB, ND * CHW], BF16, name="xqB1", tag="xqB1")
            xqB2 = pp.tile([PB, ND * 2 * CHW], BF16, name="xqB2", tag="xqB2")
            xkA = pp.tile([PB, ND * 2 * CHW], BF16, name="xkA", tag="xkA")
            xkB1 = pp.tile([PB, ND * 2 * CHW], BF16, name="xkB1", tag="xkB1")
            xkB2 = pp.tile([PB, ND * 4 * CHW], BF16, name="xkB2", tag="xkB2")
            xvA = pp.tile([PB, ND * 2 * CHW], BF16, name="xvA", tag="xvA")
            xvB1 = pp.tile([PB, ND * 2 * CHW], BF16, name="xvB1", tag="xvB1")
            xvB2 = pp.tile([PB, ND * 4 * CHW], BF16, name="xvB2", tag="xvB2")
            # projected tensors, chunked  (rows 0:64 == rows 64:128)
            qpT = [pp.tile([PB, CHW], BF16, name=f"qpT{c}", tag=f"qpT{c}")
                   for c in range(NCH)]
            kpT = [pp.tile([PB, CHW], BF16, name=f"kpT{c}", tag=f"kpT{c}")
                   for c in range(2 * NCH)]
            vpT = [pp.tile([E, CHW], BF16, name=f"vpT{c}", tag=f"vpT{c}")
                   for c in range(2 * NCH)]
            vp = [pp.tile([PB, E + 1], BF16, name=f"vp{s}", tag=f"vp{s}")
                  for s in range(NKB)]
            out_sb = pp.tile([PB, NLQ * E], F32, name="out_sb", tag="out_sb")

            def xq_ap(d, c):
                if c == 0:
                    return xqA[:, CHW * d:CHW * (d + 1)]
                if c == 1:
                    return xqB1[:, CHW * d:CHW * (d + 1)]
                w = 2 * CHW
                return xqB2[:, w * d + CHW * (c - 2):w * d + CHW * (c - 1)]

            def xk_ap(d, kc):
                w = 2 * CHW
                if kc < 2:
                    return xkA[:, w * d + CHW * kc:w * d + CHW * (kc + 1)]
                if kc < 4:
                    return xkB1[:, w * d + CHW * (kc - 2):w * d + CHW * (kc - 1)]
                w = 4 * CHW
                return xkB2[:, w * d + CHW * (kc - 4):w * d + CHW * (kc - 3)]

            def xv_ap(d, kc):
                w = 2 * CHW
                if kc < 2:
                    return xvA[:, w * d + CHW * kc:w * d + CHW * (kc + 1)]
                if kc < 4:
                    return xvB1[:, w * d + CHW * (kc - 2):w * d + CHW * (kc - 1)]
                w = 4 * CHW
                return xvB2[:, w * d + CHW * (kc - 4):w * d + CHW * (kc - 3)]

            # ---- input DMAs: by first-use time, explicit queues ----
            def dmerge(t_sb, x_d, lo, hi):
                """One DMA loading [128, ND*(hi-lo)]: all d-slices of
                DRAM cols [lo:hi] side by side."""
                return (t_sb[:].rearrange("p (d s) -> p d s", d=ND),
                        x_d[:, lo:hi].rearrange("(d p) s -> p d s", p=PB))

            dma_scalar = [
                (wq_sb[:], wq_d[:]),
                (idf_sb[:], id_d[:]),
                dmerge(xqA, xq_d, 0, CHW),
                (wv_sb[:], wv_d[:]),
                (mk_sb[:], cm_d[:]),
            ]
            dma_sync = [
                (wk_sb[:], wk_d[:]),
                dmerge(xkA, xk_d, 0, 2 * CHW),
                dmerge(xqB1, xq_d, CHW, 2 * CHW),
                dmerge(xkB1, xk_d, 2 * CHW, 4 * CHW),
                dmerge(xqB2, xq_d, 2 * CHW, 4 * CHW),
                dmerge(xkB2, xk_d, 4 * CHW, 8 * CHW),
            ]
            dma_pool = [
                dmerge(xvA, xv_d, 0, 2 * CHW),
                dmerge(xvB1, xv_d, 2 * CHW, 4 * CHW),
                dmerge(xvB2, xv_d, 4 * CHW, 8 * CHW),
            ]
            for o, inp in dma_scalar:
                nc.scalar.dma_start(out=o, in_=inp)
            for o, inp in dma_sync:
                nc.sync.dma_start(out=o, in_=inp)
            for o, inp in dma_pool:
                nc.gpsimd.dma_start(out=o, in_=inp)

            nc.vector.tensor_copy(idb_sb[:], idf_sb[:])
            for s in range(NKB):
                nc.vector.memset(vp[s][:], 1.0)   # ones column prefill

            # ---- PE warmup: keep HAM busy until real projections start ----
            for i in range(NWARM):
                wm_ps = stp.tile([PB, CHW], F32, tag="st")
                nc.tensor.matmul(wm_ps[:], wq_sb[:, 0:PB], wq_sb[:, 0:4 * PB],
                                 start=True, stop=True)

            def vtrans(s):
                """PE-transpose one projected-V block to k-major + copy out."""
                vproj(s // 4)
                vt_ps = pjp.tile([PB, E], BF16, tag="pj")
                nc.tensor.transpose(vt_ps[:], vpT[s // 4][:, PB * (s % 4):PB * (s % 4 + 1)],
                                    idb_sb[0:E, 0:E])
                nc.vector.tensor_copy(vp[s][:, 0:E], vt_ps[:])

            def project(c):
                """Project Q chunk c and K chunks 2c, 2c+1 (V projections
                are emitted later, interleaved between ST groups)."""
                qp_ps = pjp.tile([PB, CHW], F32, tag="pj")
                for d in range(ND):
                    nc.tensor.matmul(qp_ps[:], wq_sb[:, PB * d:PB * (d + 1)],
                                     xq_ap(d, c),
                                     start=(d == 0), stop=(d == ND - 1))
                nc.vector.tensor_copy(qpT[c][:], qp_ps[:])
                for kc in (2 * c, 2 * c + 1):
                    kp_ps = pjp.tile([PB, CHW], F32, tag="pj")
                    for d in range(ND):
                        nc.tensor.matmul(kp_ps[:], wk_sb[:, PB * d:PB * (d + 1)],
                                         xk_ap(d, kc),
                                         start=(d == 0), stop=(d == ND - 1))
                    nc.vector.tensor_copy(kpT[kc][:], kp_ps[:])

            vproj_done = set()

            def vproj(kc):
                """Lazily project V chunk kc (called at first vtrans use)."""
                if kc in vproj_done:
                    return
                vproj_done.add(kc)
                vq_ps = pjp.tile([E, CHW], F32, tag="pj")
                for d in range(ND):
                    nc.tensor.matmul(vq_ps[:], wv_sb[:, E * d:E * (d + 1)],
                                     xv_ap(d, kc),
                                     start=(d == 0), stop=(d == ND - 1))
                nc.vector.tensor_copy(vpT[kc][:], vq_ps[:])

            def st_mm(st_ps, ji, kb, c):
                pb = E * (kb % 2)
                kc, col = kb // 4, PB * (kb % 4)
                nc.tensor.matmul(st_ps[:, CHW * ji:CHW * (ji + 1)],
                                 kpT[kc][pb:pb + E, col:col + PB],
                                 qpT[c][pb:pb + E, :],
                                 start=True, stop=True, tile_position=(pb, 0))

            # prologue: first chunk's projections
            project(0)

            for c in range(NCH):
                nkb = 8 * c + 8
                zt_ps = ztp.tile([E + 1, CHW], F32, tag="zt")
                korder = list(range(0, nkb))
                groups = [korder[i:i + GRP] for i in range(0, nkb, GRP)]
                pend = []
                drain_state = {"n": 0}

                def drain_avs(p_et, p_kbs, nkb=nkb, zt_ps=zt_ps, c=c, ds=drain_state):
                    for kb in p_kbs:      # late vtrans, spread across groups
                        if kb >= 8 * c:
                            vtrans(kb)
                    for ji, kb in enumerate(p_kbs):
                        nc.tensor.matmul(
                            zt_ps[:], vp[kb][:],
                            p_et[:, CHW * ji:CHW * (ji + 1)],
                            start=(ds["n"] == 0),
                            stop=(ds["n"] == nkb - 1),
                            skip_group_check=True)
                        ds["n"] += 1

                for gi, kbs in enumerate(groups):
                    gw = len(kbs) * CHW
                    st_ps = stp.tile([PB, GRP * CHW], F32, tag="st")
                    # pair of consecutive kblocks -> concurrent row-tiled MMs
                    if len(kbs) >= 2:
                        st_mm(st_ps, 0, kbs[0], c)
                        st_mm(st_ps, 1, kbs[1], c)
                        rest = range(2, len(kbs))
                    else:
                        rest = range(len(kbs))
                    for ji in rest:
                        st_mm(st_ps, ji, kbs[ji], c)
                    if len(pend) > LAG - 1:
                        drain_avs(*pend.pop(0))
                    et_sb = wp.tile([PB, GRP * CHW], BF16, tag="et")
                    nc.scalar.activation(
                        et_sb[:, :gw], st_ps[:, :gw],
                        mybir.ActivationFunctionType.Exp, scale=0.125)
                    m0 = kbs[0] - 8 * c
                    if len(kbs) == 2 and m0 >= 0:
                        # both kblocks in the diag region: adjacent masks,
                        # one [128, 1024] multiply
                        nc.vector.tensor_mul(
                            et_sb[:, :gw], et_sb[:, :gw],
                            mk_sb[:, CHW * m0:CHW * (m0 + 2)])
                    else:
                        for ji, kb in enumerate(kbs):
                            m = kb - 8 * c
                            if m >= 0:
                                nc.vector.tensor_mul(
                                    et_sb[:, CHW * ji:CHW * (ji + 1)],
                                    et_sb[:, CHW * ji:CHW * (ji + 1)],
                                    mk_sb[:, CHW * m:CHW * (m + 1)])
                    pend.append((et_sb, kbs))
                for p in pend:
                    drain_avs(*p)
                zs_sb = wp.tile([E + 1, CHW], F32, tag="zs")
                nc.vector.tensor_copy(zs_sb[:], zt_ps[:])
                # project next chunk while exp/AV tail of this chunk drains
                if c + 1 < NCH:
                    project(c + 1)
                # normalize via transpose (denominator = col E)
                for j in range(4):
                    zn_ps = ztp.tile([PB, E + 1], F32, tag="zt")
                    nc.tensor.transpose(zn_ps[:], zs_sb[:, PB * j:PB * (j + 1)],
                                        idf_sb[0:E + 1, 0:E + 1])
                    rc_sb = wp.tile([PB, 1], F32, tag="rc")
                    nc.vector.reciprocal(rc_sb[:], zn_ps[:, E:E + 1])
                    jj = 4 * c + j
                    nc.vector.tensor_scalar_mul(out_sb[:, E * jj:E * (jj + 1)],
                                                zn_ps[:, 0:E], rc_sb[:])
                # chunk's output block: one contiguous DMA of [128, 4*E] f32
                nc.gpsimd.dma_start(
                    out=out_d[:, 4 * E * c:4 * E * (c + 1)],
                    in_=out_sb[:, 4 * E * c:4 * E * (c + 1)])
    nc.finalize()
    return nc


def make_core_inputs(key_np, value_np, query_np, Wk, Wv, Wq):
    """Host-side sharding: returns in_maps list of 8 dicts."""
    bf = lambda a: np.ascontiguousarray(a).astype(NPBF16)

    def pmajor(w, width):
        """[D, width] -> [128, ND*width] partition-major."""
        return np.ascontiguousarray(
            w.reshape(ND, PB, width).transpose(1, 0, 2).reshape(PB, ND * width))

    wq2 = pmajor(np.concatenate([Wq, Wq], axis=1), PB)
    wk2 = pmajor(np.concatenate([Wk, Wk], axis=1), PB)
    wv2 = pmajor(Wv, E)
    in_maps = []
    for c in range(8):
        b, h = c // 2, c % 2
        qrows = np.concatenate(
            [np.arange(PB * (2 * j + h), PB * (2 * j + h) + PB) for j in range(NLQ)])
        # causal masks: mask m applies to kblock kb = 8c+m of every chunk;
        # section jj (q sub-block) has global q-block g = 8c+2jj+h,
        # class = m - 2jj - h: <0 keep, ==0 triangular, >0 zero.
        cmask = np.zeros((8, PB, CHW), dtype=np.float32)
        ki = np.arange(PB)[:, None]
        qi = np.arange(PB)[None, :]
        tri = (ki <= qi).astype(np.float32)
        for m in range(8):
            for jj in range(4):
                cls = m - 2 * jj - h
                blk = np.ones((PB, PB), np.float32) if cls < 0 else (
                    tri if cls == 0 else np.zeros((PB, PB), np.float32))
                cmask[m][:, PB * jj:PB * (jj + 1)] = blk
        cmask_pm = np.ascontiguousarray(
            cmask.transpose(1, 0, 2).reshape(PB, 8 * CHW))
        in_maps.append({
            "xq": bf(query_np[b][qrows].T),
            "xk": bf(key_np[b].T),
            "xv": bf(value_np[b].T),
            "wq": bf(wq2), "wk": bf(wk2), "wv": bf(wv2),
            "cmask": bf(cmask_pm),
            "ident": np.eye(PB, dtype=np.float32),
        })
    return in_maps


def assemble_output(results):
    """results: list of 8 dicts with 'out' [128, 16*64] f32 -> Z [B,S,E]."""
    Z = np.zeros((B, S, E), dtype=np.float32)
    for c in range(8):
        b, h = c // 2, c % 2
        o = results[c]["out"].reshape(PB, NLQ, E)  # [p, j, e]
        for j in range(NLQ):
            g = 2 * j + h
            Z[b, PB * g:PB * (g + 1), :] = o[:, j, :]
    return Z


def kernel(key_inputs, value_inputs, query_inputs, Wk, Wv, Wq):
    from concourse.bass_utils import run_bass_kernel_spmd
    nc = build_nc()
    in_maps = make_core_inputs(np.asarray(key_inputs), np.asarray(value_inputs),
                               np.asarray(query_inputs), np.asarray(Wk),
                               np.asarray(Wv), np.asarray(Wq))
    res = run_bass_kernel_spmd(nc, in_maps, core_ids=list(range(8)))
    return assemble_output(res.results)


# revision 9
# speedup vs baseline: 1.2774x; 1.0880x over previous
"""Distributed causal attention head on 8 TRN2 NeuronCores.

Problem: B=4, S=4096, D_in=512, D_out=64 causal attention
  K/V/Q = X @ W; scores = Q@K^T (causal, /sqrt(64)); Z = softmax(scores)@V

Sharding: core c = 2*b + h handles batch b, seq-half h.
q-rows are interleaved at 128-row-block granularity (core h owns global
q-blocks {2j+h}), which makes the causal block schedule IDENTICAL on all
cores (SPMD-safe) and balances FLOPs exactly.  Every core loads the full
(transposed) K/V inputs of its batch and projects them locally.

Perf structure:
 - All host tensors are partition-major so every DMA descriptor is a
   1-8KB contiguous row (descriptor-processing, not bandwidth, limits
   badly-shaped DMAs).
 - Input DMAs are ordered by first-use time and split so each chunk's
   slices arrive just ahead of the compute that needs them; triggers are
   spread over the sync/scalar/gpsimd queues (a trigger occupies its
   queue ~0.6us and a queue's transfers serialize).
 - PE warmup spin (dummy matmuls on the weight tile) so the HAM clock
   gate is at 8/8 (2.4 GHz) when real projections start.
 - Wq/Wk are host-duplicated to [D, 128] so projections emit [128, 512]
   PSUM (both parity copies in one matmul + one CAST), feeding the
   row-tiled score matmuls directly.
 - Scores are computed transposed ST[k,q] with K=64 PAIRS row-tiled in
   the PE; exp on ACT in groups of 2 kblocks (scale=1/8 folded, no
   max-subtraction: |scores/8| < ~1.5); AV matmuls accumulate Z^T in
   PSUM with a ones-column in Vp giving the softmax denominator free;
   Z^T is PE-transposed back to q-major, normalized with a reciprocal +
   tensor_scalar_mul into a persistent [128, 16*64] output tile that is
   DMA'd per chunk (contiguous, host un-permutes).
"""

import numpy as np
import ml_dtypes

import concourse.bass as bass
import concourse.bacc as bacc
import concourse.mybir as mybir
import concourse.tile as tile

B, S, D, E = 4, 4096, 512, 64
PB = 128                      # partition block
NKB = S // PB                 # 32 k-blocks (global)
NLQ = NKB // 2                # 16 local q-blocks per core
NCH = 4                       # q-chunks of 512 per core
CHW = 512                     # q-chunk width
ND = D // PB                  # 4 d-slices
GRP = 2                       # kblocks per exp group
LAG = 4                       # ST->AV software pipeline depth (groups)
NWARM = 10                    # PE warmup matmuls
BF16 = mybir.dt.bfloat16
F8 = mybir.dt.float8e4
F32 = mybir.dt.float32
NPBF16 = ml_dtypes.bfloat16
NPF8 = ml_dtypes.float8_e4m3fn
WSCALE = 16.0


def build_nc():
    nc = bacc.Bacc(None)

    xq_d = nc.declare_dram_parameter("xq", [D, S // 2], F8, isOutput=False)
    xk_d = nc.declare_dram_parameter("xk", [D, S], F8, isOutput=False)
    xv_d = nc.declare_dram_parameter("xv", [D, S], BF16, isOutput=False)
    wq_d = nc.declare_dram_parameter("wq", [PB, ND * PB], F8, isOutput=False)
    wk_d = nc.declare_dram_parameter("wk", [PB, ND * PB], F8, isOutput=False)
    wv_d = nc.declare_dram_parameter("wv", [PB, ND * E], BF16, isOutput=False)
    cm_d = nc.declare_dram_parameter("cmask", [PB, 8 * CHW], BF16, isOutput=False)
    id_d = nc.declare_dram_parameter("ident", [PB, PB], F32, isOutput=False)
    out_d = nc.declare_dram_parameter("out", [PB, NLQ * E], F32, isOutput=True)

    with tile.TileContext(nc) as tc:
        with tc.tile_pool(name="persist", bufs=1) as pp, \
             tc.tile_pool(name="st_ps", bufs=2, space="PSUM") as stp, \
             tc.tile_pool(name="pj_ps", bufs=2, space="PSUM") as pjp, \
             tc.tile_pool(name="zt_ps", bufs=2, space="PSUM") as ztp, \
             tc.tile_pool(name="work", bufs=2 * LAG + 2) as wp:
            # ---- persistent SBUF tiles ----
            wq_sb = pp.tile([PB, ND * PB], F8, name="wq_sb", tag="wq_sb")
            wk_sb = pp.tile([PB, ND * PB], F8, name="wk_sb", tag="wk_sb")
            wv_sb = pp.tile([PB, ND * E], BF16, name="wv_sb", tag="wv_sb")
            mk_sb = pp.tile([PB, 8 * CHW], BF16, name="mk_sb", tag="mk_sb")
            idf_sb = pp.tile([PB, PB], F32, name="idf_sb", tag="idf_sb")
            idb_sb = pp.tile([PB, PB], BF16, name="idb_sb", tag="idb_sb")
            # inputs, split by first-use time (A: chunk 0, B1: chunk 1,
            # B2: chunks 2-3 / k-chunks 4-7); each tile holds all 4
            # d-slices side by side so one DMA covers it.
            xqA = pp.tile([PB, ND * CHW], F8, name="xqA", tag="xqA")
            xqB1 = pp.tile([PB, ND * CHW], F8, name="xqB1", tag="xqB1")
            xqB2 = pp.tile([PB, ND * 2 * CHW], F8, name="xqB2", tag="xqB2")
            xkA = pp.tile([PB, ND * 2 * CHW], F8, name="xkA", tag="xkA")
            xkB1 = pp.tile([PB, ND * 2 * CHW], F8, name="xkB1", tag="xkB1")
            xkB2 = pp.tile([PB, ND * 4 * CHW], F8, name="xkB2", tag="xkB2")
            xvA = pp.tile([PB, ND * 2 * CHW], BF16, name="xvA", tag="xvA")
            xvB1 = pp.tile([PB, ND * 2 * CHW], BF16, name="xvB1", tag="xvB1")
            xvB2 = pp.tile([PB, ND * 4 * CHW], BF16, name="xvB2", tag="xvB2")
            # projected tensors, chunked  (rows 0:64 == rows 64:128)
            qpT = [pp.tile([PB, CHW], BF16, name=f"qpT{c}", tag=f"qpT{c}")
                   for c in range(NCH)]
            kpT = [pp.tile([PB, CHW], BF16, name=f"kpT{c}", tag=f"kpT{c}")
                   for c in range(2 * NCH)]
            vpT = [pp.tile([E, CHW], BF16, name=f"vpT{c}", tag=f"vpT{c}")
                   for c in range(2 * NCH)]
            vp = [pp.tile([PB, E + 1], BF16, name=f"vp{s}", tag=f"vp{s}")
                  for s in range(NKB)]
            out_sb = pp.tile([PB, NLQ * E], F32, name="out_sb", tag="out_sb")

            def xq_ap(d, c):
                if c == 0:
                    return xqA[:, CHW * d:CHW * (d + 1)]
                if c == 1:
                    return xqB1[:, CHW * d:CHW * (d + 1)]
                w = 2 * CHW
                return xqB2[:, w * d + CHW * (c - 2):w * d + CHW * (c - 1)]

            def xk_ap(d, kc):
                w = 2 * CHW
                if kc < 2:
                    return xkA[:, w * d + CHW * kc:w * d + CHW * (kc + 1)]
                if kc < 4:
                    return xkB1[:, w * d + CHW * (kc - 2):w * d + CHW * (kc - 1)]
                w = 4 * CHW
                return xkB2[:, w * d + CHW * (kc - 4):w * d + CHW * (kc - 3)]

            def xv_ap(d, kc):
                w = 2 * CHW
                if kc < 2:
                    return xvA[:, w * d + CHW * kc:w * d + CHW * (kc + 1)]
                if kc < 4:
                    return xvB1[:, w * d + CHW * (kc - 2):w * d + CHW * (kc - 1)]
                w = 4 * CHW
                return xvB2[:, w * d + CHW * (kc - 4):w * d + CHW * (kc - 3)]

            # ---- input DMAs: by first-use time, explicit queues ----
            def dmerge(t_sb, x_d, lo, hi):
                """One DMA loading [128, ND*(hi-lo)]: all d-slices of
                DRAM cols [lo:hi] side by side."""
                return (t_sb[:].rearrange("p (d s) -> p d s", d=ND),
                        x_d[:, lo:hi].rearrange("(d p) s -> p d s", p=PB))

            dma_scalar = [
                (wq_sb[:], wq_d[:]),
                dmerge(xqA, xq_d, 0, CHW),
                (idf_sb[:], id_d[:]),
                (wv_sb[:], wv_d[:]),
                (mk_sb[:, 0:4 * CHW], cm_d[:, 0:4 * CHW]),
                (mk_sb[:, 4 * CHW:8 * CHW], cm_d[:, 4 * CHW:8 * CHW]),
            ]
            dma_sync = [
                (wk_sb[:], wk_d[:]),
                dmerge(xkA, xk_d, 0, 2 * CHW),
                dmerge(xqB1, xq_d, CHW, 2 * CHW),
                dmerge(xkB1, xk_d, 2 * CHW, 4 * CHW),
                dmerge(xqB2, xq_d, 2 * CHW, 4 * CHW),
                dmerge(xkB2, xk_d, 4 * CHW, 8 * CHW),
            ]
            dma_pool = [
                dmerge(xvA, xv_d, 0, 2 * CHW),
                dmerge(xvB1, xv_d, 2 * CHW, 4 * CHW),
                dmerge(xvB2, xv_d, 4 * CHW, 8 * CHW),
            ]
            for o, inp in dma_scalar:
                nc.scalar.dma_start(out=o, in_=inp)
            for o, inp in dma_sync:
                nc.sync.dma_start(out=o, in_=inp)
            for o, inp in dma_pool:
                nc.gpsimd.dma_start(out=o, in_=inp)

            nc.vector.tensor_copy(idb_sb[:], idf_sb[:])
            for s in range(NKB):
                nc.vector.memset(vp[s][:], 1.0)   # ones column prefill

            # ---- PE warmup: keep HAM busy until real projections start ----
            for i in range(NWARM):
                wm_ps = stp.tile([PB, CHW], F32, tag="st")
                nc.tensor.matmul(wm_ps[:], wq_sb[:, 0:PB], wq_sb[:, 0:4 * PB],
                                 start=True, stop=True)

            def vtrans(s):
                """PE-transpose one projected-V block to k-major + copy out."""
                vproj(s // 4)
                vt_ps = pjp.tile([PB, E], BF16, tag="pj")
                nc.tensor.transpose(vt_ps[:], vpT[s // 4][:, PB * (s % 4):PB * (s % 4 + 1)],
                                    idb_sb[0:E, 0:E])
                nc.vector.tensor_copy(vp[s][:, 0:E], vt_ps[:])

            def project(c):
                """Project Q chunk c and K chunks 2c, 2c+1 (V projections
                are emitted later, interleaved between ST groups)."""
                qp_ps = pjp.tile([PB, CHW], F32, tag="pj")
                for d in range(ND):
                    nc.tensor.matmul(qp_ps[:], wq_sb[:, PB * d:PB * (d + 1)],
                                     xq_ap(d, c),
                                     start=(d == 0), stop=(d == ND - 1))
                nc.vector.tensor_copy(qpT[c][:], qp_ps[:])
                for kc in (2 * c, 2 * c + 1):
                    kp_ps = pjp.tile([PB, CHW], F32, tag="pj")
                    for d in range(ND):
                        nc.tensor.matmul(kp_ps[:], wk_sb[:, PB * d:PB * (d + 1)],
                                         xk_ap(d, kc),
                                         start=(d == 0), stop=(d == ND - 1))
                    nc.vector.tensor_copy(kpT[kc][:], kp_ps[:])

            vproj_done = set()

            def vproj(kc):
                """Lazily project V chunk kc (called at first vtrans use)."""
                if kc in vproj_done:
                    return
                vproj_done.add(kc)
                vq_ps = pjp.tile([E, CHW], F32, tag="pj")
                for d in range(ND):
                    nc.tensor.matmul(vq_ps[:], wv_sb[:, E * d:E * (d + 1)],
                                     xv_ap(d, kc),
                                     start=(d == 0), stop=(d == ND - 1))
                nc.vector.tensor_copy(vpT[kc][:], vq_ps[:])

            def st_mm(st_ps, ji, kb, c):
                pb = E * (kb % 2)
                kc, col = kb // 4, PB * (kb % 4)
                nc.tensor.matmul(st_ps[:, CHW * ji:CHW * (ji + 1)],
                                 kpT[kc][pb:pb + E, col:col + PB],
                                 qpT[c][pb:pb + E, :],
                                 start=True, stop=True, tile_position=(pb, 0))

            # prologue: first chunk's projections
            project(0)

            for c in range(NCH):
                nkb = 8 * c + 8
                zt_ps = ztp.tile([E + 1, CHW], F32, tag="zt")
                korder = list(range(0, nkb))
                groups = [korder[i:i + GRP] for i in range(0, nkb, GRP)]
                pend = []
                drain_state = {"n": 0}

                def drain_avs(p_et, p_kbs, nkb=nkb, zt_ps=zt_ps, c=c, ds=drain_state):
                    for kb in p_kbs:      # late vtrans, spread across groups
                        if kb >= 8 * c:
                            vtrans(kb)
                    for ji, kb in enumerate(p_kbs):
                        nc.tensor.matmul(
                            zt_ps[:], vp[kb][:],
                            p_et[:, CHW * ji:CHW * (ji + 1)],
                            start=(ds["n"] == 0),
                            stop=(ds["n"] == nkb - 1),
                            skip_group_check=True)
                        ds["n"] += 1

                for gi, kbs in enumerate(groups):
                    gw = len(kbs) * CHW
                    st_ps = stp.tile([PB, GRP * CHW], F32, tag="st")
                    # pair of consecutive kblocks -> concurrent row-tiled MMs
                    if len(kbs) >= 2:
                        st_mm(st_ps, 0, kbs[0], c)
                        st_mm(st_ps, 1, kbs[1], c)
                        rest = range(2, len(kbs))
                    else:
                        rest = range(len(kbs))
                    for ji in rest:
                        st_mm(st_ps, ji, kbs[ji], c)
                    if len(pend) > LAG - 1:
                        drain_avs(*pend.pop(0))
                    et_sb = wp.tile([PB, GRP * CHW], BF16, tag="et")
                    nc.scalar.activation(
                        et_sb[:, :gw], st_ps[:, :gw],
                        mybir.ActivationFunctionType.Exp, scale=0.125 / (WSCALE * WSCALE))
                    m0 = kbs[0] - 8 * c
                    if len(kbs) == 2 and m0 >= 0:
                        # both kblocks in the diag region: adjacent masks,
                        # one [128, 1024] multiply
                        nc.vector.tensor_mul(
                            et_sb[:, :gw], et_sb[:, :gw],
                            mk_sb[:, CHW * m0:CHW * (m0 + 2)])
                    else:
                        for ji, kb in enumerate(kbs):
                            m = kb - 8 * c
                            if m >= 0:
                                nc.vector.tensor_mul(
                                    et_sb[:, CHW * ji:CHW * (ji + 1)],
                                    et_sb[:, CHW * ji:CHW * (ji + 1)],
                                    mk_sb[:, CHW * m:CHW * (m + 1)])
                    pend.append((et_sb, kbs))
                for p in pend:
                    drain_avs(*p)
                zs_sb = wp.tile([E + 1, CHW], F32, tag="zs")
                nc.vector.tensor_copy(zs_sb[:], zt_ps[:])
                # project next chunk while exp/AV tail of this chunk drains
                if c + 1 < NCH:
                    project(c + 1)
                # normalize via transpose (denominator = col E)
                for j in range(4):
                    zn_ps = ztp.tile([PB, E + 1], F32, tag="zt")
                    nc.tensor.transpose(zn_ps[:], zs_sb[:, PB * j:PB * (j + 1)],
                                        idf_sb[0:E + 1, 0:E + 1])
                    rc_sb = wp.tile([PB, 1], F32, tag="rc")
                    nc.vector.reciprocal(rc_sb[:], zn_ps[:, E:E + 1])
                    jj = 4 * c + j
                    nc.vector.tensor_scalar_mul(out_sb[:, E * jj:E * (jj + 1)],
                                                zn_ps[:, 0:E], rc_sb[:])
                # chunk's output block: one contiguous DMA of [128, 4*E] f32
                nc.gpsimd.dma_start(
                    out=out_d[:, 4 * E * c:4 * E * (c + 1)],
                    in_=out_sb[:, 4 * E * c:4 * E * (c + 1)])
    nc.finalize()
    return nc


def make_core_inputs(key_np, value_np, query_np, Wk, Wv, Wq):
    """Host-side sharding: returns in_maps list of 8 dicts."""
    bf = lambda a: np.ascontiguousarray(a).astype(NPBF16)

    def pmajor(w, width):
        """[D, width] -> [128, ND*width] partition-major."""
        return np.ascontiguousarray(
            w.reshape(ND, PB, width).transpose(1, 0, 2).reshape(PB, ND * width))

    f8 = lambda a: np.ascontiguousarray(a).astype(NPF8)
    Wqs, Wks = Wq * WSCALE, Wk * WSCALE
    wq2 = f8(pmajor(np.concatenate([Wqs, Wqs], axis=1), PB))
    wk2 = f8(pmajor(np.concatenate([Wks, Wks], axis=1), PB))
    wv2 = pmajor(Wv, E)
    in_maps = []
    for c in range(8):
        b, h = c // 2, c % 2
        qrows = np.concatenate(
            [np.arange(PB * (2 * j + h), PB * (2 * j + h) + PB) for j in range(NLQ)])
        # causal masks: mask m applies to kblock kb = 8c+m of every chunk;
        # section jj (q sub-block) has global q-block g = 8c+2jj+h,
        # class = m - 2jj - h: <0 keep, ==0 triangular, >0 zero.
        cmask = np.zeros((8, PB, CHW), dtype=np.float32)
        ki = np.arange(PB)[:, None]
        qi = np.arange(PB)[None, :]
        tri = (ki <= qi).astype(np.float32)
        for m in range(8):
            for jj in range(4):
                cls = m - 2 * jj - h
                blk = np.ones((PB, PB), np.float32) if cls < 0 else (
                    tri if cls == 0 else np.zeros((PB, PB), np.float32))
                cmask[m][:, PB * jj:PB * (jj + 1)] = blk
        cmask_pm = np.ascontiguousarray(
            cmask.transpose(1, 0, 2).reshape(PB, 8 * CHW))
        in_maps.append({
            "xq": f8(query_np[b][qrows].T),
            "xk": f8(key_np[b].T),
            "xv": bf(value_np[b].T),
            "wq": wq2, "wk": wk2, "wv": bf(wv2),
            "cmask": bf(cmask_pm),
            "ident": np.eye(PB, dtype=np.float32),
        })
    return in_maps


def assemble_output(results):
    """results: list of 8 dicts with 'out' [128, 16*64] f32 -> Z [B,S,E]."""
    Z = np.zeros((B, S, E), dtype=np.float32)
    for c in range(8):
        b, h = c // 2, c % 2
        o = results[c]["out"].reshape(PB, NLQ, E)  # [p, j, e]
        for j in range(NLQ):
            g = 2 * j + h
            Z[b, PB * g:PB * (g + 1), :] = o[:, j, :]
    return Z


def kernel(key_inputs, value_inputs, query_inputs, Wk, Wv, Wq):
    from concourse.bass_utils import run_bass_kernel_spmd
    nc = build_nc()
    in_maps = make_core_inputs(np.asarray(key_inputs), np.asarray(value_inputs),
                               np.asarray(query_inputs), np.asarray(Wk),
                               np.asarray(Wv), np.asarray(Wq))
    res = run_bass_kernel_spmd(nc, in_maps, core_ids=list(range(8)))
    return assemble_output(res.results)
